# revision 23
# baseline (speedup 1.0000x reference)
"""Radius-graph KNN (nn_Distance) Trainium2 Bass kernel.

Problem: N=12288 atoms in 96 contiguous batches of 128 (batch vector is
sorted), radius r=5, K=32 nearest in-radius neighbors per node, self-loop
padding.  The distance matrix is block-diagonal: each 128-atom block only
interacts with itself.  Sharding: 8 cores x 12 blocks.

Numerics are bit-matched to the jax reference (axon platform):
  - G2 = (2*pos) @ pos.T on the PE (fp32) == XLA dot, exactly
  - sq / t1 / key chain replicates the reference rounding order
  - 4 rounds of (max8, max_index, match_replace) == stable lax.top_k
  - per-partition reorder via gpsimd local_scatter (rank inversion + u16
    pairs); invalid slots drop out via negative scatter indices
"""

import numpy as np

N = 12288
B = 96
BLK = 128
K = 32
N_CORES = 8
BLOCKS_PER_CORE = B // N_CORES  # 12
R2 = 25.0

_compiled = {}


def _build_bass(nt: int):
    import concourse.bacc as bacc
    import concourse.mybir as mybir
    from concourse.tile import TileContext

    f32 = mybir.dt.float32
    i16 = mybir.dt.int16
    i32 = mybir.dt.int32
    u16 = mybir.dt.uint16
    u8 = mybir.dt.uint8
    Alu = mybir.AluOpType
    Act = mybir.ActivationFunctionType
    X = mybir.AxisListType.X

    NTB = nt * BLK        # 1536 nodes per core
    NTK = nt * K          # 384 edge slots per partition-row

    nc = bacc.Bacc("TRN2", target_bir_lowering=False, debug=False,
                   num_devices=N_CORES)

    posT_d = nc.dram_tensor("posT", [3, NTB], f32, kind="ExternalInput")
    posr_d = nc.dram_tensor("posr", [BLK, nt * 3], f32, kind="ExternalInput")
    posf_d = nc.dram_tensor("posf", [1, nt * 3 * BLK], f32, kind="ExternalInput")
    eye_d = nc.dram_tensor("eye", [BLK, BLK], f32, kind="ExternalInput")
    iota2_d = nc.dram_tensor("iota2", [BLK, K], f32, kind="ExternalInput")
    iotap_d = nc.dram_tensor("iotap", [BLK, 1], f32, kind="ExternalInput")
    srcc_d = nc.dram_tensor("srcc", [BLK, NTK], i32, kind="ExternalInput")

    dst_d = nc.dram_tensor("dst", [BLK, NTK], i32, kind="ExternalOutput")
    src_d = nc.dram_tensor("src", [BLK, NTK], i32, kind="ExternalOutput")
    w_d = nc.dram_tensor("w", [BLK, NTK], f32, kind="ExternalOutput")
    vec_d = nc.dram_tensor("vec", [BLK, 3 * NTK], f32, kind="ExternalOutput")

    with TileContext(nc) as tc:
        with tc.tile_pool(name="const", bufs=1) as cpool, \
             tc.tile_pool(name="big", bufs=1) as big, \
             tc.tile_pool(name="sb", bufs=4) as sb, \
             tc.tile_pool(name="ps", bufs=2, space="PSUM") as ps, \
             tc.tile_pool(name="ps1", bufs=2, space="PSUM") as ps1:
            eye = cpool.tile_from(eye_d[:, :])
            iota2 = cpool.tile_from(iota2_d[:, :])      # 2,4,...,64 per row
            iotap = cpool.tile_from(iotap_d[:, :])      # partition index p
            neg1 = cpool.tile([1, BLK], f32)
            nc.vector.memset(neg1[:, :], -1.0)
            one1 = cpool.tile([1, BLK], f32)
            nc.vector.memset(one1[:, :], 1.0)
            bz = cpool.tile([BLK, 1], f32)
            nc.vector.memset(bz[:, :], 0.0)

            # ---- whole-shard loads ----------------------------------------
            posT_all = big.tile([3, NTB], f32)
            nc.sync.dma_start(posT_all[:, :], posT_d[:, :])
            posr_all = big.tile([BLK, nt * 3], f32)
            nc.sync.dma_start(posr_all[:, :], posr_d[:, :])
            posf_all = big.tile([1, nt * 3 * BLK], f32)
            nc.sync.dma_start(posf_all[:, :], posf_d[:, :])
            srcc = big.tile([BLK, NTK], i32)
            nc.sync.dma_start(srcc[:, :], srcc_d[:, :])

            # ---- batched prologue -----------------------------------------
            posT2_all = big.tile([3, NTB], f32)
            nc.scalar.activation(posT2_all[:, :], posT_all[:, :], Act.Identity,
                                 bias=bz[0:3, 0:1], scale=2.0)
            prsq = big.tile([BLK, nt * 3], f32)
            nc.vector.tensor_mul(prsq[:, :], posr_all[:, :], posr_all[:, :])
            sqcol_all = big.tile([BLK, nt], f32)
            nc.vector.reduce_sum(sqcol_all[:, :],
                                 prsq[:, :].rearrange("p (t c) -> p t c", c=3),
                                 axis=X)
            # per-tile transpose of sqcol -> [1,128] rows (keeps each tile's
            # chain independent; avoids a serializing sb2sb DMA)
            sqrow_all = big.tile([1, NTB], f32)
            for t in range(nt):
                srp = ps1.tile([1, BLK], f32, tag="sqrowT")
                nc.tensor.transpose(srp[:, :], sqcol_all[:, t:t + 1], eye[:, :])
                nc.scalar.copy(sqrow_all[0:1, t * BLK:(t + 1) * BLK], srp[:, :])

            # persistent per-shard working tensors
            maxv_all = big.tile([BLK, NTK], f32)
            idx_all = big.tile([BLK, NTK], u16)
            planes_all = big.tile([BLK, nt * 3 * BLK], f32)
            rank_all = big.tile([BLK, nt * BLK], i16)
            data1_all = big.tile([BLK, NTK], i16)
            idxs2_all = big.tile([BLK, nt * 2 * BLK], i16)
            vg_all = big.tile([BLK, 3 * NTK], f32)
            valid_all = big.tile([BLK, NTK], u8)

            # src passthrough (no deps -- overlaps everything)
            nc.sync.dma_start(src_d[:, :], srcc[:, :])

            import os
            GROUP = int(os.environ.get("KNN_GROUP", "12"))
            IL_ENGINE = os.environ.get("KNN_IL", "dve")
            ngroups = nt // GROUP

            def emit_epilogue(g):
                t0, t1g = g * GROUP, (g + 1) * GROUP
                kslc = slice(t0 * K, t1g * K)
                # vec interleave (t,c,k) -> (t,k,c) on ACT, then DMA
                vil = vec_il[:, :].rearrange("p (t k c) -> p t k c",
                                             t=nt, c=3)
                vga = vg_all[:, :].rearrange("p (t c k) -> p t c k",
                                             t=nt, c=3)
                for d in range(3):
                    if IL_ENGINE == "act":
                        nc.scalar.activation(vil[:, t0:t1g, :, d],
                                             vga[:, t0:t1g, d, :],
                                             Act.Identity, bias=bz[:, 0:1],
                                             scale=1.0)
                    elif IL_ENGINE == "pool":
                        nc.gpsimd.tensor_copy(vil[:, t0:t1g, :, d],
                                              vga[:, t0:t1g, d, :])
                    else:
                        nc.vector.tensor_copy(vil[:, t0:t1g, :, d],
                                              vga[:, t0:t1g, d, :])
                vslc = slice(t0 * 3 * K, t1g * 3 * K)
                nc.sync.dma_start(vec_d[:, vslc], vec_il[:, vslc])
                # weight
                nc.scalar.square(vsq[:, vslc], vec_il[:, vslc])
                nc.vector.reduce_sum(
                    ss[:, kslc],
                    vsq[:, vslc].rearrange("p (tk c) -> p tk c", c=3),
                    axis=X)
                nc.scalar.sqrt(w[:, kslc], ss[:, kslc])
                nc.sync.dma_start(w_d[:, kslc], w[:, kslc])
                # dst (core-local): valid ? j : p
                nc.scalar.activation(idxf[:, kslc], idx_all[:, kslc],
                                     Act.Identity, bias=bz[:, 0:1], scale=1.0)
                nc.vector.tensor_copy(
                    dstf[:, kslc],
                    iotap[:, 0:1].to_broadcast([BLK, GROUP * K]))
                nc.vector.copy_predicated(dstf[:, kslc], valid_all[:, kslc],
                                          idxf[:, kslc])
                nc.scalar.activation(dsti[:, kslc], dstf[:, kslc],
                                     Act.Identity, bias=bz[:, 0:1], scale=1.0)
                nc.sync.dma_start(dst_d[:, kslc], dsti[:, kslc])

            vec_il = big.tile([BLK, 3 * NTK], f32)
            vsq = big.tile([BLK, 3 * NTK], f32)
            ss = big.tile([BLK, NTK], f32)
            w = big.tile([BLK, NTK], f32)
            idxf = big.tile([BLK, NTK], f32)
            dstf = big.tile([BLK, NTK], f32)
            dsti = big.tile([BLK, NTK], i32)

            # ---- per-tile phases, software-pipelined in pairs -------------
            PAIR = int(os.environ.get("KNN_PAIR", "1"))

            def emit_key(t):
                sqb_ps = ps.tile([BLK, BLK], f32, tag="sqb_ps")
                nc.tensor.matmul(sqb_ps[:, :], one1[:, :],
                                 sqrow_all[0:1, t * BLK:(t + 1) * BLK],
                                 start=True, stop=True)
                t1 = sb.tile([BLK, BLK], f32, tag="t1")
                nc.scalar.activation(t1[:, :], sqb_ps[:, :], Act.Identity,
                                     bias=sqcol_all[:, t:t + 1], scale=1.0)
                g2_ps = ps.tile([BLK, BLK], f32, tag="g2_ps")
                nc.tensor.matmul(g2_ps[:, :],
                                 posT2_all[:, t * BLK:(t + 1) * BLK],
                                 posT_all[:, t * BLK:(t + 1) * BLK],
                                 start=True, stop=True)
                key = sb.tile([BLK, BLK], f32, tag=f"key{t % PAIR}")
                nc.vector.tensor_sub(key[:, :], g2_ps[:, :], t1[:, :])
                return key

            def emit_post(t):
                maxv = maxv_all[:, t * K:(t + 1) * K]
                idx = idx_all[:, t * K:(t + 1) * K]
                valid = valid_all[:, t * K:(t + 1) * K]
                nc.vector.tensor_scalar(valid, maxv, -R2, None, op0=Alu.is_ge)
                data1 = data1_all[:, t * K:(t + 1) * K]
                nc.vector.tensor_mul(data1, valid, iota2[:, :])
                rank = rank_all[:, t * BLK:(t + 1) * BLK]
                nc.gpsimd.local_scatter(rank, data1, idx.bitcast(i16),
                                        channels=BLK, num_elems=BLK,
                                        num_idxs=K)
                idxs2 = idxs2_all[:, t * 2 * BLK:(t + 1) * 2 * BLK]
                nc.gpsimd.tensor_scalar(idxs2[0:BLK, 0:2 * BLK:2], rank, -2.0,
                                        None, op0=Alu.add)
                nc.gpsimd.tensor_scalar(idxs2[0:BLK, 1:2 * BLK:2], rank, -1.0,
                                        None, op0=Alu.add)
                nxb_ps = ps.tile([BLK, 3 * BLK], f32, tag="nxb_ps")
                nc.tensor.matmul(nxb_ps[:, :], neg1[:, :],
                                 posf_all[0:1, t * 3 * BLK:(t + 1) * 3 * BLK],
                                 start=True, stop=True)
                planes = planes_all[:, t * 3 * BLK:(t + 1) * 3 * BLK]
                for d in range(3):
                    nc.scalar.activation(
                        planes[0:BLK, d * BLK:(d + 1) * BLK],
                        nxb_ps[:, :].rearrange("p (j c) -> p c j", c=3)[:, d, :],
                        Act.Identity,
                        bias=posr_all[:, 3 * t + d:3 * t + d + 1], scale=1.0)
                vg = vg_all[:, t * 3 * K:(t + 1) * 3 * K]
                for d in range(3):
                    nc.gpsimd.local_scatter(
                        vg[0:BLK, d * K:(d + 1) * K].bitcast(u16),
                        planes[0:BLK, d * BLK:(d + 1) * BLK].bitcast(u16),
                        idxs2, channels=BLK, num_elems=2 * K,
                        num_idxs=2 * BLK)

            for tp in range(0, nt, PAIR):
                ts_ = list(range(tp, min(tp + PAIR, nt)))
                keys = {t: emit_key(t) for t in ts_}
                # interleave the selection rounds of the pair on DVE
                for r in range(4):
                    for t in ts_:
                        nc.vector.max(
                            out=maxv_all[:, t * K + r * 8:t * K + (r + 1) * 8],
                            in_=keys[t][:, :])
                    for t in ts_:
                        nc.vector.max_index(
                            out=idx_all[:, t * K + r * 8:t * K + (r + 1) * 8],
                            in_max=maxv_all[:, t * K + r * 8:t * K + (r + 1) * 8],
                            in_values=keys[t][:, :])
                    if r < 3:
                        for t in ts_:
                            nc.vector.match_replace(
                                out=keys[t][:, :],
                                in_to_replace=maxv_all[:, t * K + r * 8:
                                                       t * K + (r + 1) * 8],
                                in_values=keys[t][:, :], imm_value=-1e30)
                for t in ts_:
                    emit_post(t)
                for t in ts_:
                    if t % GROUP == GROUP - 1:
                        emit_epilogue(t // GROUP)

    nc.compile()
    return nc


def _get_compiled(nt: int):
    if nt not in _compiled:
        _compiled[nt] = _build_bass(nt)
    return _compiled[nt]


def _make_in_maps(pos: np.ndarray):
    nt = BLOCKS_PER_CORE
    eye = np.eye(BLK, dtype=np.float32)
    iota2 = np.broadcast_to(2.0 * np.arange(1, K + 1, dtype=np.float32),
                            (BLK, K)).copy()
    iotap = np.arange(BLK, dtype=np.float32).reshape(BLK, 1).copy()

    in_maps = []
    for c in range(N_CORES):
        base = c * nt * BLK
        shard = pos[base:base + nt * BLK]                     # [nt*128, 3]
        blocks = shard.reshape(nt, BLK, 3)
        posT = np.ascontiguousarray(shard.T)                  # [3, nt*128]
        posr = np.ascontiguousarray(
            blocks.transpose(1, 0, 2).reshape(BLK, nt * 3))   # [128, (t c)]
        posf = np.ascontiguousarray(shard.reshape(1, -1))     # [1, nt*384]
        # src constant: src[p, (t k)] = base + t*128 + p
        tt = np.arange(nt, dtype=np.int32)[None, :, None]
        pp = np.arange(BLK, dtype=np.int32)[:, None, None]
        srcc = np.ascontiguousarray(
            np.broadcast_to(base + tt * BLK + pp, (BLK, nt, K))
            .reshape(BLK, nt * K)).astype(np.int32)
        in_maps.append(dict(posT=posT, posr=posr, posf=posf, eye=eye,
                            iota2=iota2, iotap=iotap, srcc=srcc))
    return in_maps


def _run_device(pos: np.ndarray):
    from concourse import bass_utils

    nc = _get_compiled(BLOCKS_PER_CORE)
    in_maps = _make_in_maps(pos)
    return bass_utils.run_bass_kernel_spmd(nc, in_maps,
                                           core_ids=list(range(N_CORES)))


def _assemble(results):
    nt = BLOCKS_PER_CORE
    dst_l, src_l, w_l, vec_l = [], [], [], []
    tt = np.arange(nt, dtype=np.int32)[None, :, None]
    for c, r in enumerate(results):
        base = c * nt * BLK
        # device layouts are [p, t, k]; reorder to [(t p), k]
        d = r["dst"].reshape(BLK, nt, K) + base + tt * BLK
        dst_l.append(np.ascontiguousarray(d.transpose(1, 0, 2)).reshape(-1, K))
        s = r["src"].reshape(BLK, nt, K)
        src_l.append(np.ascontiguousarray(s.transpose(1, 0, 2)).reshape(-1, K))
        wv = r["w"].reshape(BLK, nt, K)
        w_l.append(np.ascontiguousarray(wv.transpose(1, 0, 2)).reshape(-1, K))
        v = r["vec"].reshape(BLK, nt, K, 3)
        vec_l.append(np.ascontiguousarray(v.transpose(1, 0, 2, 3))
                     .reshape(-1, K, 3))
    dst = np.concatenate(dst_l, 0)
    src = np.concatenate(src_l, 0)
    w = np.concatenate(w_l, 0)
    vec = np.concatenate(vec_l, 0)
    edge_index = np.stack([src.reshape(-1), dst.reshape(-1)]).astype(np.int32)
    return (edge_index, w.reshape(-1).astype(np.float32),
            vec.reshape(-1, 3).astype(np.float32))


def _numpy_fallback(pos, batch):
    """Generic correct (not bitwise) implementation for unexpected inputs."""
    pos = np.asarray(pos, np.float32)
    batch = np.asarray(batch, np.int32)
    n = pos.shape[0]
    sq = (pos.astype(np.float64) ** 2).sum(-1)
    d2 = sq[:, None] + sq[None, :] - 2.0 * (
        pos.astype(np.float64) @ pos.T.astype(np.float64))
    d2 = np.maximum(d2, 0.0)
    valid = (batch[:, None] == batch[None, :]) & (d2 <= R2)
    masked = np.where(valid, d2, np.inf)
    order = np.argsort(masked, axis=1, kind="stable")[:, :K]
    vals = np.take_along_axis(masked, order, axis=1)
    row = np.broadcast_to(np.arange(n, dtype=np.int32)[:, None], (n, K))
    col = np.where(np.isfinite(vals), order.astype(np.int32), row)
    src = row.reshape(-1)
    dstv = col.reshape(-1)
    edge_index = np.stack([src, dstv]).astype(np.int32)
    ev = pos[src] - pos[dstv]
    m = src != dstv
    ssv = (ev * ev).sum(-1)
    ew = np.where(m, np.sqrt(np.where(m, ssv, 1.0)), 0.0).astype(np.float32)
    return edge_index, ew, ev


def kernel(pos, batch):
    pos = np.ascontiguousarray(np.asarray(pos, dtype=np.float32))
    batch = np.asarray(batch, dtype=np.int32)

    expected_batch = np.repeat(np.arange(B, dtype=np.int32), N // B)
    if pos.shape != (N, 3) or not np.array_equal(batch, expected_batch):
        return _numpy_fallback(pos, batch)

    res = _run_device(pos)
    return _assemble(res.results)
